# revision 25
# baseline (speedup 1.0000x reference)
"""Trainium2 Bass kernel for nn_BiasVectorsBlock (MVN sampling block).

Computes, for x [32, 2048, 512] and z [32, 512]:
    mean = mean(x, axis=(0,1))
    cov  = mean_b( xc_b^T xc_b / (T-1) ),  xc_b = x_b - mean_t(x_b)
    L    = cholesky(cov);  out = mean + z @ L^T

Strategy (8 NeuronCores, data-parallel over B):
  - core c DMA-loads its 4 batches once (f32), casts to bf16 on
    DVE/ScalarE.  TensorE accumulates upper-triangle Gram strips in
    PSUM across all 64 chunks, plus tiny per-strip indicator matmuls
    (rhs = one-hot batch columns) that give the per-batch column sums
    s_b on PE for free — no DVE fold chains.
  - tail: transpose s blocks via eye-matmuls, apply the -s_b s_b^T/T
    corrections, subtract SHIFT*I, pack upper strips + transposed mean
    into a [128, 1288] bf16 payload.
  - AllReduce = hand-rolled XOR recursive doubling over remote_dma
    SBUF->SBUF broadcasts (relative dests, so the SPMD program is
    uniform): 3 stages (partner = me ^ 2^k), each stage sends the
    current partial (sliced across dest slots so all DMA engines run),
    waits for the partner's payload, adds it on DVE.  Replaces the
    ncfw collective (24us + 11.5us trigger delay) with ~10us.
  - every core then runs the sqrt-free Cholesky fixed-point rounds
    (Y <- Phi_u(E - Y^T Y), 2 rounds) and the affine out = z + z@Y +
    mean; core 0's output is the result.
"""

import os
import sys

for _p in ("/opt/trn_rl_repo",):
    if _p not in sys.path and os.path.isdir(_p):
        sys.path.insert(0, _p)

import numpy as np

B, T, D = 32, 2048, 512
NCORES = 8
BC = B // NCORES          # batches per core
CH = T // 128             # 128-row chunks per batch
DENOM = (T - 1) * B       # cov denominator
SHIFT = DENOM / NCORES    # identity shift per core, so payload is zero-mean
W = [512, 384, 256, 128]  # upper-strip widths (strip i: rows 128i.., cols 128i..512)
OFFS = [0, 512, 896, 1152]  # packed strip offsets
PAY = 1288                # payload cols: 1280 strips + 4 mean(T) + 4 pad
NSL = PAY // 4            # 322 cols (644 B) per slice


def _build_nc():
    import concourse.bacc as bacc
    import concourse.mybir as mybir
    import ml_dtypes
    from bass_rust import InstructionNameOrderedSet
    from concourse import library_config
    from concourse.tile import TileContext

    f32 = mybir.dt.float32
    bf16 = mybir.dt.bfloat16
    mult = mybir.AluOpType.mult

    nc = bacc.Bacc(None, num_devices=NCORES)

    x_in = nc.declare_dram_parameter("x", [BC, T, D], f32, isOutput=False)
    z_in = nc.declare_dram_parameter("z", [B, D], f32, isOutput=False)
    zt_in = nc.declare_dram_parameter("zt", [D, B], f32, isOutput=False)
    out_ext = nc.declare_dram_parameter("out", [B, D], f32, isOutput=True)

    # ---- constants (embedded in the NEFF) ----
    # -Phi mask: local cols 0:128 = diagonal block (strict-upper -> -1,
    # diag -> -0.5, lower -> 0); cols 128:512 -> -1.
    m = np.zeros((128, 512), np.float32)
    m[:, 128:] = -1.0
    r, c = np.indices((128, 128))
    m[:, :128] = np.where(c > r, -1.0, np.where(c == r, -0.5, 0.0)).astype(np.float32)
    maskneg_d = nc.inline_tensor(m, name="maskneg")
    maskpd_d = nc.inline_tensor(-m * (2.0 ** -16), name="maskpd")

    eye = np.eye(128, dtype=np.float32)
    eyeb_d = nc.inline_tensor((-eye * 2.0 ** -16).astype(ml_dtypes.bfloat16), name="eyeb")
    eye128b_d = nc.inline_tensor(eye.astype(ml_dtypes.bfloat16), name="eye128b")
    negshifti_d = nc.inline_tensor((-SHIFT) * eye, name="negshifti")
    # indicator columns: col 4b+j = 1 iff j == b (slice [:, 4b:4b+4] per batch)
    ind = np.zeros((128, 4 * BC), np.float32)
    for b in range(BC):
        ind[:, 4 * b + b] = 1.0
    ind16_d = nc.inline_tensor(ind.astype(ml_dtypes.bfloat16), name="ind16")
    ones4x1_d = nc.inline_tensor(np.ones((BC, 1), ml_dtypes.bfloat16), name="ones4x1")
    # mean-broadcast selectors: bc4[k, 32j+b] = 2^-16 iff k == j
    bc = np.zeros((BC, 4 * B), np.float32)
    for j in range(4):
        bc[j, 32 * j:32 * (j + 1)] = 2.0 ** -16
    bc4_d = nc.inline_tensor(bc.astype(ml_dtypes.bfloat16), name="bc4")

    with TileContext(nc) as tc, \
            tc.tile_pool(name="sb", bufs=1) as sb, \
            tc.tile_pool(name="dr", space="DRAM", bufs=1) as dr:

        # Dummy 64-byte ncfw AllReduce: its presence makes the runtime
        # initialize the collectives communicator and gate execution start
        # on all 8 cores being loaded, which kills the multi-ms launch
        # skew between cores (the hand-rolled exchange otherwise inherits
        # it as dead wait time).  Runs on the CC cores concurrently with
        # phase A; nobody reads the result.
        ccz = sb.tile([1, 16], f32, name="ccz")
        nc.vector.memset(ccz[:, :], 0.0)
        cc_in = dr.tile([1, 16], f32, name="cc_in")
        cc_out = dr.tile([1, 16], f32, addr_space="Shared", name="cc_out")
        nc.scalar.dma_start(out=cc_in[:, :], in_=ccz[:, :])
        nc.gpsimd.collective_compute(
            "AllReduce",
            mybir.AluOpType.add,
            replica_groups=[list(range(NCORES))],
            ins=[cc_in[:, :].opt()],
            outs=[cc_out[:, :].opt()],
        )

        # AR buffers + sems up front so descriptor-gen preps can be emitted
        # early (Pool sequencer does ~1.1us of desc-gen per prep).
        accs = [sb.tile([128, PAY], bf16, name=f"acc{k}") for k in range(4)]
        rbufs = [sb.tile([128, PAY], bf16, name=f"r{k}") for k in range(3)]
        rsems = [nc.alloc_semaphore(f"rsem{k}") for k in range(3)]
        psems = [nc.alloc_semaphore(f"psem{k}") for k in range(3)]
        lsem = nc.alloc_semaphore("lsem")

        # One broadcast per stage with the partner repeated in every legal
        # slot: the ucode lane map merges same-dest slots into one transfer
        # carrying the whole payload on all the slots' engines (16 for the
        # intra-die stages, the 8 D2D-capable ones for the cross-die stage).
        # One ring entry + one trigger per stage keeps the SWDGE ring and
        # Pool sequencer off the critical path.
        nc.gpsimd.load_library(library_config.remote_dma)
        for k in range(3):
            delta = 1 << k
            if delta < 4:
                rd = [(0, delta)] + [None] * 7
            else:
                rd = [None] * 4 + [(0, delta)] + [None] * 3
            nc.gpsimd.remote_dma_broadcast(
                out_ap=rbufs[k][:, :],
                in_ap=accs[k][:, :],
                remote_sem=rsems[k], local_sem=lsem, rdests=rd)

        # ---- phase A: Gram strips + per-batch column sums ----
        with tc.tile_pool(name="psA", space="PSUM", bufs=1) as ps:
            g = [ps.tile([128, W[i]], f32, tag=f"g{i}", bufs=1, name=f"g{i}")
                 for i in range(4)]
            sblk16 = ps.tile([128, 4 * BC], f32, tag="sb16", bufs=1, name="sblk16")
            sblk = [sblk16[:, 4 * i:4 * (i + 1)] for i in range(4)]
            for b in range(BC):
                xf = sb.tile([128, CH * D], f32, tag="xf", bufs=4, name=f"xf{b}")
                xb = sb.tile([128, CH * D], bf16, tag="xb", bufs=2, name=f"xb{b}")
                xf3 = xf.rearrange("p (c d) -> p c d", d=D)
                xs3 = x_in[b].rearrange("(c p) d -> p c d", p=128)
                if b == 0:
                    c0 = 0
                    for span in (2, 2, 4, 4, 4):
                        nc.sync.dma_start(out=xf3[:, c0:c0 + span, :],
                                          in_=xs3[:, c0:c0 + span, :])
                        c0 += span
                elif b < BC - 1:
                    half = CH // 2
                    nc.sync.dma_start(out=xf3[:, :half, :], in_=xs3[:, :half, :])
                    nc.sync.dma_start(out=xf3[:, half:, :], in_=xs3[:, half:, :])
                else:
                    # last batch: quarter DMAs so the casts (and the last
                    # chunk matmuls) chase the tail of the stream instead
                    # of waiting for a whole half-batch.
                    q4 = CH // 4
                    for qi in range(4):
                        nc.sync.dma_start(out=xf3[:, qi * q4:(qi + 1) * q4, :],
                                          in_=xs3[:, qi * q4:(qi + 1) * q4, :])
                if b == 0:
                    # only ind16 (needed by the first chunk matmuls) and
                    # z/zt load now; the big masks wait until the x DMA
                    # stream is wound down (b == 2) so they don't steal
                    # HBM bandwidth from the critical path.
                    ind16 = sb.tile_from(ind16_d[:, :], name="ind16_sb", forced_dma_engine=mybir.EngineType.Activation)
                    z_sb = sb.tile([B, D], f32, name="z_sb")
                    nc.scalar.dma_start(out=z_sb[:, :], in_=z_in[:, :])
                    zts = []
                    for k in range(4):
                        zt_k = sb.tile([128, B], f32, name=f"zt{k}_sb")
                        nc.scalar.dma_start(out=zt_k[:, :],
                                            in_=zt_in[k * 128:(k + 1) * 128, :])
                        ztb_k = sb.tile([128, B], bf16, name=f"ztb{k}_sb")
                        nc.vector.tensor_copy(out=ztb_k[:, :], in_=zt_k[:, :])
                        zts.append(ztb_k)
                    nc.vector.memset(accs[0][:, 1284:PAY], 0.0)
                if b == 2:
                    maskneg = sb.tile_from(maskneg_d[:, :], name="maskneg_sb", forced_dma_engine=mybir.EngineType.Activation)
                    maskpd = sb.tile_from(maskpd_d[:, :], name="maskpd_sb", forced_dma_engine=mybir.EngineType.Activation)
                    eyeb = sb.tile_from(eyeb_d[:, :], name="eyeb_sb", forced_dma_engine=mybir.EngineType.Activation)
                    eye128b = sb.tile_from(eye128b_d[:, :], name="eye128b_sb", forced_dma_engine=mybir.EngineType.Activation)
                    negshifti = sb.tile_from(negshifti_d[:, :], name="negshifti_sb", forced_dma_engine=mybir.EngineType.Activation)
                    ones4x1 = sb.tile_from(ones4x1_d[:, :], name="ones4x1_sb", forced_dma_engine=mybir.EngineType.Activation)
                    bc4 = sb.tile_from(bc4_d[:, :], name="bc4_sb", forced_dma_engine=mybir.EngineType.Activation)
                # casts alternate DVE/ScalarE
                if b == 0:
                    c0 = 0
                    for pi, span in enumerate((2, 2, 4, 4, 4)):
                        so = xb[:, c0 * D:(c0 + span) * D]
                        si = xf[:, c0 * D:(c0 + span) * D]
                        if pi % 2 == 0:
                            nc.vector.tensor_copy(out=so, in_=si)
                        else:
                            nc.scalar.copy(out=so, in_=si)
                        c0 += span
                else:
                    q = CH * D // 4
                    for qi in range(4):
                        so = xb[:, qi * q:(qi + 1) * q]
                        si = xf[:, qi * q:(qi + 1) * q]
                        if qi % 2 == 0:
                            nc.vector.tensor_copy(out=so, in_=si)
                        else:
                            nc.scalar.copy(out=so, in_=si)
                first = b == 0
                last = b == BC - 1
                for cch in range(CH):
                    xc = xb[:, cch * D:(cch + 1) * D]
                    st = first and cch == 0
                    sp = last and cch == CH - 1
                    for i in range(4):
                        nc.tensor.matmul(
                            g[i][:, :],
                            lhsT=xc[:, i * 128:(i + 1) * 128],
                            rhs=xc[:, 128 * i:],
                            start=st, stop=False,
                        )
                        nc.tensor.matmul(
                            sblk[i],
                            lhsT=xc[:, i * 128:(i + 1) * 128],
                            rhs=ind16[:, 4 * b:4 * (b + 1)],
                            start=st, stop=sp,
                        )

            # ---- tail: s extraction, corrections, mean, pack ----
            sblk_sb = []
            for i in range(4):
                t = sb.tile([128, BC], bf16, name=f"sblk_sb{i}")
                if i % 2 == 0:
                    nc.vector.tensor_copy(out=t[:, :], in_=sblk[i])
                else:
                    nc.scalar.copy(out=t[:, :], in_=sblk[i])
                sblk_sb.append(t)
            srowT = ps.tile([BC, D], f32, tag="srowT", bufs=1, name="srowT")
            for i in range(4):
                nc.tensor.matmul(
                    srowT[:, 128 * i:128 * (i + 1)],
                    lhsT=sblk_sb[i][:, :], rhs=eye128b[:, :],
                    start=True, stop=True,
                )
            s_bf = sb.tile([BC, D], bf16, name="s_bf")
            nc.vector.tensor_copy(out=s_bf[:, :], in_=srowT[:, :])
            sneg = sb.tile([BC, D], bf16, name="sneg")
            nc.vector.tensor_scalar_mul(sneg[:, :], srowT[:, :], -1.0 / T)
            for i in range(4):
                nc.tensor.matmul(
                    g[i][:, :],
                    lhsT=sneg[:, i * 128:(i + 1) * 128],
                    rhs=s_bf[:, 128 * i:],
                    start=False, stop=True,
                )
            # transposed mean columns: mc[:, j] = sum_b s_b[128j:128(j+1)]
            mc = ps.tile([128, 4], f32, tag="mc", bufs=1, name="mc")
            for j in range(4):
                nc.tensor.matmul(
                    mc[:, j:j + 1],
                    lhsT=s_bf[:, 128 * j:128 * (j + 1)], rhs=ones4x1[:, :],
                    start=True, stop=True,
                )
            # pack into acc0
            pack_ops = []
            for i in range(4):
                pack_ops.append(nc.vector.tensor_add(
                    out=accs[0][:, OFFS[i]:OFFS[i] + 128],
                    in0=g[i][:, 0:128],
                    in1=negshifti[:, :],
                ))
                if W[i] > 128:
                    pack_ops.append(nc.scalar.copy(
                        out=accs[0][:, OFFS[i] + 128:OFFS[i] + W[i]],
                        in_=g[i][:, 128:W[i]],
                    ))
            pack_ops.append(nc.vector.tensor_copy(out=accs[0][:, 1280:1284],
                                                  in_=mc[:, :]))
            si = nc.vector.sem_inc(psems[0], 1)
            si.ins.add_sync_dependencies_from(
                InstructionNameOrderedSet([op.ins.name for op in pack_ops]))

        # ---- AllReduce: XOR recursive doubling over remote_dma ----
        # Trigger k waits on psem k (incremented by an explicit sem_inc
        # ordered after the producers of acc_k).  The rsem waits sit
        # directly on the adds and are emitted with value 0 so Tile's
        # single-core scheduling sim (which cannot see the partner's
        # remote sem updates) does not flag a deadlock; the real wait
        # values are patched in after scheduling, before finalize.
        patch_waits = []
        for k in range(3):
            nc.gpsimd.trigger_dma(count=1)._wait_ge(psems[k], 1)
            add = nc.vector.tensor_add(out=accs[k + 1][:, :],
                                       in0=accs[k][:, :],
                                       in1=rbufs[k][:, :])
            add._wait_ge(rsems[k], 0)
            patch_waits.append((add.ins, f"rsem{k}", 2))
            if k + 1 < 3:
                si = nc.vector.sem_inc(psems[k + 1], 1)
                si.ins.add_sync_dependencies_from(
                    InstructionNameOrderedSet([add.ins.name]))

        # ---- phase B: Cholesky fixed-point iteration + affine ----
        acc3 = accs[3]
        with tc.tile_pool(name="psB", space="PSUM", bufs=1) as ps:
            ebn_raw = [acc3[:, OFFS[i]:OFFS[i] + W[i]] for i in range(4)]
            # round 0: Y = Phi(E) = raw * (mask/DENOM)
            Y = []
            for i in range(4):
                y0 = sb.tile([128, W[i]], bf16, tag="y", bufs=8, name=f"y0_{i}")
                nc.vector.tensor_tensor(out=y0[:, :], in0=ebn_raw[i],
                                        in1=maskpd[:, :W[i]], op=mult)
                Y.append(y0)
            # round 1: Y <- Phi(E - Y^T Y)
            newY = []
            for i in range(4):
                p = ps.tile([128, W[i]], f32, tag="it", bufs=4, name=f"it1_{i}")
                first = True
                for k in range(i + 1):
                    lo = 128 * (i - k)
                    nc.tensor.matmul(
                        p[:, :],
                        lhsT=Y[k][:, lo:lo + 128],
                        rhs=Y[k][:, lo:],
                        start=first, stop=False,
                    )
                    first = False
                nc.tensor.matmul(p[:, :], lhsT=eyeb[:, :], rhs=ebn_raw[i],
                                 start=first, stop=True)
                ny = sb.tile([128, W[i]], bf16, tag="y", bufs=8, name=f"y1_{i}")
                nc.vector.tensor_tensor(out=ny[:, :], in0=p[:, :],
                                        in1=maskneg[:, :W[i]], op=mult)
                newY.append(ny)
            Y = newY

            # affine: out = z + z @ Y + mean
            aff = ps.tile([B, D], f32, tag="aff", bufs=1, name="aff")
            for k in range(4):
                nc.tensor.matmul(
                    aff[:, 128 * k:],
                    lhsT=zts[k][:, :],
                    rhs=Y[k][:, :],
                    start=(k == 0), stop=False,
                )
            # mean: transpose mc columns back to a [4, 128] row block, then
            # broadcast to all 32 output rows (selector consts carry 1/(B*T)).
            mrowT = ps.tile([BC, 128], f32, tag="mrowT", bufs=1, name="mrowT")
            nc.tensor.matmul(mrowT[:, :], lhsT=acc3[:, 1280:1284],
                             rhs=eye128b[:, :], start=True, stop=True)
            m4 = sb.tile([BC, 128], bf16, name="m4")
            nc.vector.tensor_copy(out=m4[:, :], in_=mrowT[:, :])
            for j in range(4):
                nc.tensor.matmul(
                    aff[:, 128 * j:128 * (j + 1)],
                    lhsT=bc4[:, 32 * j:32 * (j + 1)],
                    rhs=m4[:, :],
                    start=False, stop=True,
                )
            out_sb = sb.tile([B, D], f32, name="out_sb")
            nc.vector.tensor_add(out=out_sb[:, :], in0=aff[:, :], in1=z_sb[:, :])
            nc.scalar.dma_start(out=out_ext[:, :], in_=out_sb[:, :])

    # patch the real remote-sem wait values in (see comment at emission)
    for ins, semname, val in patch_waits:
        si = ins.sync_info
        found = False
        for w in si.on_wait:
            if w.ant_name == semname:
                w.wait_value = val
                found = True
        assert found, (semname, si)
        ins.sync_info = si

    nc.finalize()
    return nc


_NC_CACHE = {}


def _get_nc():
    if "nc" not in _NC_CACHE:
        _NC_CACHE["nc"] = _build_nc()
    return _NC_CACHE["nc"]


def _in_maps(x, z):
    zt = np.ascontiguousarray(z.T)
    return [
        {"x": np.ascontiguousarray(x[c * BC:(c + 1) * BC]), "z": z, "zt": zt}
        for c in range(NCORES)
    ]


def kernel(x: np.ndarray, z: np.ndarray) -> np.ndarray:
    from concourse.bass_utils import run_bass_kernel_spmd

    x = np.ascontiguousarray(np.asarray(x, dtype=np.float32))
    z = np.ascontiguousarray(np.asarray(z, dtype=np.float32))
    nc = _get_nc()
    res = run_bass_kernel_spmd(nc, _in_maps(x, z), core_ids=list(range(NCORES)))
    return np.asarray(res.results[0]["out"], dtype=np.float32)


# revision 27
# speedup vs baseline: 1.0492x; 1.0492x over previous
"""Trainium2 Bass kernel for nn_BiasVectorsBlock (MVN sampling block).

Computes, for x [32, 2048, 512] and z [32, 512]:
    mean = mean(x, axis=(0,1))
    cov  = mean_b( xc_b^T xc_b / (T-1) ),  xc_b = x_b - mean_t(x_b)
    L    = cholesky(cov);  out = mean + z @ L^T

Strategy (8 NeuronCores, data-parallel over B):
  - core c DMA-loads its 4 batches once (f32), casts to bf16 on
    DVE/ScalarE.  TensorE accumulates upper-triangle Gram strips in
    PSUM across all 64 chunks, plus tiny per-strip indicator matmuls
    (rhs = one-hot batch columns) that give the per-batch column sums
    s_b on PE for free — no DVE fold chains.
  - tail: transpose s blocks via eye-matmuls, apply the -s_b s_b^T/T
    corrections, subtract SHIFT*I, pack upper strips + transposed mean
    into a [128, 1288] bf16 payload.
  - AllReduce = hand-rolled XOR recursive doubling over remote_dma
    SBUF->SBUF broadcasts (relative dests, so the SPMD program is
    uniform): 3 stages (partner = me ^ 2^k), each stage sends the
    current partial (sliced across dest slots so all DMA engines run),
    waits for the partner's payload, adds it on DVE.  Replaces the
    ncfw collective (24us + 11.5us trigger delay) with ~10us.
  - every core then runs the sqrt-free Cholesky fixed-point rounds
    (Y <- Phi_u(E - Y^T Y), 2 rounds) and the affine out = z + z@Y +
    mean; core 0's output is the result.
"""

import os
import sys

for _p in ("/opt/trn_rl_repo",):
    if _p not in sys.path and os.path.isdir(_p):
        sys.path.insert(0, _p)

import numpy as np

B, T, D = 32, 2048, 512
NCORES = 8
BC = B // NCORES          # batches per core
CH = T // 128             # 128-row chunks per batch
DENOM = (T - 1) * B       # cov denominator
SHIFT = DENOM / NCORES    # identity shift per core, so payload is zero-mean
W = [512, 384, 256, 128]  # upper-strip widths (strip i: rows 128i.., cols 128i..512)
OFFS = [0, 512, 896, 1152]  # packed strip offsets
PAY = 1288                # payload cols: 1280 strips + 4 mean(T) + 4 pad
NSL = PAY // 4            # 322 cols (644 B) per slice


def _build_nc():
    import concourse.bacc as bacc
    import concourse.mybir as mybir
    import ml_dtypes
    from bass_rust import InstructionNameOrderedSet
    from concourse import library_config
    from concourse.tile import TileContext

    f32 = mybir.dt.float32
    bf16 = mybir.dt.bfloat16
    mult = mybir.AluOpType.mult

    nc = bacc.Bacc(None, num_devices=NCORES)

    x_in = nc.declare_dram_parameter("x", [BC, T, D], f32, isOutput=False)
    z_in = nc.declare_dram_parameter("z", [B, D], f32, isOutput=False)
    zt_in = nc.declare_dram_parameter("zt", [D, B], f32, isOutput=False)
    out_ext = nc.declare_dram_parameter("out", [B, D], f32, isOutput=True)

    # ---- constants (embedded in the NEFF) ----
    # -Phi mask: local cols 0:128 = diagonal block (strict-upper -> -1,
    # diag -> -0.5, lower -> 0); cols 128:512 -> -1.
    m = np.zeros((128, 512), np.float32)
    m[:, 128:] = -1.0
    r, c = np.indices((128, 128))
    m[:, :128] = np.where(c > r, -1.0, np.where(c == r, -0.5, 0.0)).astype(np.float32)
    maskneg_d = nc.inline_tensor(m, name="maskneg")
    maskpd_d = nc.inline_tensor(-m * (2.0 ** -16), name="maskpd")

    eye = np.eye(128, dtype=np.float32)
    eyeb_d = nc.inline_tensor((-eye * 2.0 ** -16).astype(ml_dtypes.bfloat16), name="eyeb")
    eye128b_d = nc.inline_tensor(eye.astype(ml_dtypes.bfloat16), name="eye128b")
    negshifti_d = nc.inline_tensor((-SHIFT) * eye, name="negshifti")
    # indicator columns: col 4b+j = 1 iff j == b (slice [:, 4b:4b+4] per batch)
    ind = np.zeros((128, 4 * BC), np.float32)
    for b in range(BC):
        ind[:, 4 * b + b] = 1.0
    ind16_d = nc.inline_tensor(ind.astype(ml_dtypes.bfloat16), name="ind16")
    ones4x1_d = nc.inline_tensor(np.ones((BC, 1), ml_dtypes.bfloat16), name="ones4x1")
    # mean-broadcast selectors: bc4[k, 32j+b] = 2^-16 iff k == j
    bc = np.zeros((BC, 4 * B), np.float32)
    for j in range(4):
        bc[j, 32 * j:32 * (j + 1)] = 2.0 ** -16
    bc4_d = nc.inline_tensor(bc.astype(ml_dtypes.bfloat16), name="bc4")

    with TileContext(nc) as tc, \
            tc.tile_pool(name="sb", bufs=1) as sb, \
            tc.tile_pool(name="dr", space="DRAM", bufs=1) as dr:

        # Dummy 64-byte ncfw AllReduce: its presence makes the runtime
        # initialize the collectives communicator and gate execution start
        # on all 8 cores being loaded, which kills the multi-ms launch
        # skew between cores (the hand-rolled exchange otherwise inherits
        # it as dead wait time).  Runs on the CC cores concurrently with
        # phase A; nobody reads the result.
        ccz = sb.tile([1, 16], f32, name="ccz")
        nc.vector.memset(ccz[:, :], 0.0)
        cc_in = dr.tile([1, 16], f32, name="cc_in")
        cc_out = dr.tile([1, 16], f32, addr_space="Shared", name="cc_out")
        nc.scalar.dma_start(out=cc_in[:, :], in_=ccz[:, :])
        nc.gpsimd.collective_compute(
            "AllReduce",
            mybir.AluOpType.add,
            replica_groups=[list(range(NCORES))],
            ins=[cc_in[:, :].opt()],
            outs=[cc_out[:, :].opt()],
        )

        # AR buffers + sems up front so descriptor-gen preps can be emitted
        # early (Pool sequencer does ~1.1us of desc-gen per prep).
        accs = [sb.tile([128, PAY], bf16, name=f"acc{k}") for k in range(4)]
        rbufs = [sb.tile([128, PAY], bf16, name=f"r{k}") for k in range(3)]
        rsems = [nc.alloc_semaphore(f"rsem{k}") for k in range(3)]
        psems = [nc.alloc_semaphore(f"psem{k}") for k in range(3)]
        lsem = nc.alloc_semaphore("lsem")

        # One broadcast per stage with the partner repeated in every legal
        # slot: the ucode lane map merges same-dest slots into one transfer
        # carrying the whole payload on all the slots' engines (16 for the
        # intra-die stages, the 8 D2D-capable ones for the cross-die stage).
        # One ring entry + one trigger per stage keeps the SWDGE ring and
        # Pool sequencer off the critical path.
        nc.gpsimd.load_library(library_config.remote_dma)
        for k in range(3):
            delta = 1 << k
            if delta < 4:
                rd = [(0, delta)] + [None] * 7
            else:
                rd = [None] * 4 + [(0, delta)] + [None] * 3
            nc.gpsimd.remote_dma_broadcast(
                out_ap=rbufs[k][:, :],
                in_ap=accs[k][:, :],
                remote_sem=rsems[k], local_sem=lsem, rdests=rd)

        # ---- phase A: Gram strips + per-batch column sums ----
        with tc.tile_pool(name="psA", space="PSUM", bufs=1) as ps:
            g = [ps.tile([128, W[i]], f32, tag=f"g{i}", bufs=1, name=f"g{i}")
                 for i in range(4)]
            sblk16 = ps.tile([128, 4 * BC], f32, tag="sb16", bufs=1, name="sblk16")
            sblk = [sblk16[:, 4 * i:4 * (i + 1)] for i in range(4)]
            for b in range(BC):
                xf = sb.tile([128, CH * D], f32, tag="xf", bufs=4, name=f"xf{b}")
                xb = sb.tile([128, CH * D], bf16, tag="xb", bufs=2, name=f"xb{b}")
                xf3 = xf.rearrange("p (c d) -> p c d", d=D)
                xs3 = x_in[b].rearrange("(c p) d -> p c d", p=128)
                if b == 0:
                    c0 = 0
                    for span in (2, 2, 4, 4, 4):
                        nc.sync.dma_start(out=xf3[:, c0:c0 + span, :],
                                          in_=xs3[:, c0:c0 + span, :])
                        c0 += span
                else:
                    half = CH // 2
                    nc.sync.dma_start(out=xf3[:, :half, :], in_=xs3[:, :half, :])
                    nc.sync.dma_start(out=xf3[:, half:, :], in_=xs3[:, half:, :])
                if b == 0:
                    # only ind16 (needed by the first chunk matmuls) and
                    # z/zt load now; the big masks wait until the x DMA
                    # stream is wound down (b == 2) so they don't steal
                    # HBM bandwidth from the critical path.
                    ind16 = sb.tile_from(ind16_d[:, :], name="ind16_sb", forced_dma_engine=mybir.EngineType.Activation)
                    z_sb = sb.tile([B, D], f32, name="z_sb")
                    nc.scalar.dma_start(out=z_sb[:, :], in_=z_in[:, :])
                    zts = []
                    for k in range(4):
                        zt_k = sb.tile([128, B], f32, name=f"zt{k}_sb")
                        nc.scalar.dma_start(out=zt_k[:, :],
                                            in_=zt_in[k * 128:(k + 1) * 128, :])
                        ztb_k = sb.tile([128, B], bf16, name=f"ztb{k}_sb")
                        nc.vector.tensor_copy(out=ztb_k[:, :], in_=zt_k[:, :])
                        zts.append(ztb_k)
                    nc.vector.memset(accs[0][:, 1284:PAY], 0.0)
                if b == 2:
                    maskneg = sb.tile_from(maskneg_d[:, :], name="maskneg_sb", forced_dma_engine=mybir.EngineType.Activation)
                    maskpd = sb.tile_from(maskpd_d[:, :], name="maskpd_sb", forced_dma_engine=mybir.EngineType.Activation)
                    eyeb = sb.tile_from(eyeb_d[:, :], name="eyeb_sb", forced_dma_engine=mybir.EngineType.Activation)
                    eye128b = sb.tile_from(eye128b_d[:, :], name="eye128b_sb", forced_dma_engine=mybir.EngineType.Activation)
                    negshifti = sb.tile_from(negshifti_d[:, :], name="negshifti_sb", forced_dma_engine=mybir.EngineType.Activation)
                    ones4x1 = sb.tile_from(ones4x1_d[:, :], name="ones4x1_sb", forced_dma_engine=mybir.EngineType.Activation)
                    bc4 = sb.tile_from(bc4_d[:, :], name="bc4_sb", forced_dma_engine=mybir.EngineType.Activation)
                # casts alternate DVE/ScalarE
                if b == 0:
                    c0 = 0
                    for pi, span in enumerate((2, 2, 4, 4, 4)):
                        so = xb[:, c0 * D:(c0 + span) * D]
                        si = xf[:, c0 * D:(c0 + span) * D]
                        if pi % 2 == 0:
                            nc.vector.tensor_copy(out=so, in_=si)
                        else:
                            nc.scalar.copy(out=so, in_=si)
                        c0 += span
                else:
                    q = CH * D // 4
                    for qi in range(4):
                        so = xb[:, qi * q:(qi + 1) * q]
                        si = xf[:, qi * q:(qi + 1) * q]
                        if qi % 2 == 0:
                            nc.vector.tensor_copy(out=so, in_=si)
                        else:
                            nc.scalar.copy(out=so, in_=si)
                first = b == 0
                last = b == BC - 1
                for cch in range(CH):
                    xc = xb[:, cch * D:(cch + 1) * D]
                    st = first and cch == 0
                    sp = last and cch == CH - 1
                    for i in range(4):
                        nc.tensor.matmul(
                            g[i][:, :],
                            lhsT=xc[:, i * 128:(i + 1) * 128],
                            rhs=xc[:, 128 * i:],
                            start=st, stop=False,
                        )
                        nc.tensor.matmul(
                            sblk[i],
                            lhsT=xc[:, i * 128:(i + 1) * 128],
                            rhs=ind16[:, 4 * b:4 * (b + 1)],
                            start=st, stop=sp,
                        )

            # ---- tail: s extraction, corrections, mean, pack ----
            sblk_sb = []
            for i in range(4):
                t = sb.tile([128, BC], bf16, name=f"sblk_sb{i}")
                if i % 2 == 0:
                    nc.vector.tensor_copy(out=t[:, :], in_=sblk[i])
                else:
                    nc.scalar.copy(out=t[:, :], in_=sblk[i])
                sblk_sb.append(t)
            srowT = ps.tile([BC, D], f32, tag="srowT", bufs=1, name="srowT")
            for i in range(4):
                nc.tensor.matmul(
                    srowT[:, 128 * i:128 * (i + 1)],
                    lhsT=sblk_sb[i][:, :], rhs=eye128b[:, :],
                    start=True, stop=True,
                )
            s_bf = sb.tile([BC, D], bf16, name="s_bf")
            nc.vector.tensor_copy(out=s_bf[:, :], in_=srowT[:, :])
            sneg = sb.tile([BC, D], bf16, name="sneg")
            nc.vector.tensor_scalar_mul(sneg[:, :], srowT[:, :], -1.0 / T)
            for i in range(4):
                nc.tensor.matmul(
                    g[i][:, :],
                    lhsT=sneg[:, i * 128:(i + 1) * 128],
                    rhs=s_bf[:, 128 * i:],
                    start=False, stop=True,
                )
            # transposed mean columns: mc[:, j] = sum_b s_b[128j:128(j+1)]
            mc = ps.tile([128, 4], f32, tag="mc", bufs=1, name="mc")
            for j in range(4):
                nc.tensor.matmul(
                    mc[:, j:j + 1],
                    lhsT=s_bf[:, 128 * j:128 * (j + 1)], rhs=ones4x1[:, :],
                    start=True, stop=True,
                )
            # pack into acc0
            pack_ops = []
            for i in range(4):
                pack_ops.append(nc.vector.tensor_add(
                    out=accs[0][:, OFFS[i]:OFFS[i] + 128],
                    in0=g[i][:, 0:128],
                    in1=negshifti[:, :],
                ))
                if W[i] > 128:
                    pack_ops.append(nc.scalar.copy(
                        out=accs[0][:, OFFS[i] + 128:OFFS[i] + W[i]],
                        in_=g[i][:, 128:W[i]],
                    ))
            pack_ops.append(nc.vector.tensor_copy(out=accs[0][:, 1280:1284],
                                                  in_=mc[:, :]))
            si = nc.vector.sem_inc(psems[0], 1)
            si.ins.add_sync_dependencies_from(
                InstructionNameOrderedSet([op.ins.name for op in pack_ops]))

        # ---- AllReduce: XOR recursive doubling over remote_dma ----
        # Trigger k waits on psem k (incremented by an explicit sem_inc
        # ordered after the producers of acc_k).  The rsem waits sit
        # directly on the adds and are emitted with value 0 so Tile's
        # single-core scheduling sim (which cannot see the partner's
        # remote sem updates) does not flag a deadlock; the real wait
        # values are patched in after scheduling, before finalize.
        patch_waits = []
        for k in range(3):
            nc.gpsimd.trigger_dma(count=1)._wait_ge(psems[k], 1)
            add = nc.vector.tensor_add(out=accs[k + 1][:, :],
                                       in0=accs[k][:, :],
                                       in1=rbufs[k][:, :])
            add._wait_ge(rsems[k], 0)
            patch_waits.append((add.ins, f"rsem{k}", 2))
            if k + 1 < 3:
                si = nc.vector.sem_inc(psems[k + 1], 1)
                si.ins.add_sync_dependencies_from(
                    InstructionNameOrderedSet([add.ins.name]))

        # ---- phase B: Cholesky fixed-point iteration + affine ----
        acc3 = accs[3]
        with tc.tile_pool(name="psB", space="PSUM", bufs=1) as ps:
            ebn_raw = [acc3[:, OFFS[i]:OFFS[i] + W[i]] for i in range(4)]
            # round 0: Y = Phi(E) = raw * (mask/DENOM)
            Y = []
            for i in range(4):
                y0 = sb.tile([128, W[i]], bf16, tag="y", bufs=8, name=f"y0_{i}")
                nc.vector.tensor_tensor(out=y0[:, :], in0=ebn_raw[i],
                                        in1=maskpd[:, :W[i]], op=mult)
                Y.append(y0)
            # round 1: Y <- Phi(E - Y^T Y)
            newY = []
            for i in range(4):
                p = ps.tile([128, W[i]], f32, tag="it", bufs=4, name=f"it1_{i}")
                first = True
                for k in range(i + 1):
                    lo = 128 * (i - k)
                    nc.tensor.matmul(
                        p[:, :],
                        lhsT=Y[k][:, lo:lo + 128],
                        rhs=Y[k][:, lo:],
                        start=first, stop=False,
                    )
                    first = False
                nc.tensor.matmul(p[:, :], lhsT=eyeb[:, :], rhs=ebn_raw[i],
                                 start=first, stop=True)
                ny = sb.tile([128, W[i]], bf16, tag="y", bufs=8, name=f"y1_{i}")
                nc.vector.tensor_tensor(out=ny[:, :], in0=p[:, :],
                                        in1=maskneg[:, :W[i]], op=mult)
                newY.append(ny)
            Y = newY

            # affine: out = z + z @ Y + mean
            aff = ps.tile([B, D], f32, tag="aff", bufs=1, name="aff")
            for k in range(4):
                nc.tensor.matmul(
                    aff[:, 128 * k:],
                    lhsT=zts[k][:, :],
                    rhs=Y[k][:, :],
                    start=(k == 0), stop=False,
                )
            # mean: transpose mc columns back to a [4, 128] row block, then
            # broadcast to all 32 output rows (selector consts carry 1/(B*T)).
            mrowT = ps.tile([BC, 128], f32, tag="mrowT", bufs=1, name="mrowT")
            nc.tensor.matmul(mrowT[:, :], lhsT=acc3[:, 1280:1284],
                             rhs=eye128b[:, :], start=True, stop=True)
            m4 = sb.tile([BC, 128], bf16, name="m4")
            nc.vector.tensor_copy(out=m4[:, :], in_=mrowT[:, :])
            for j in range(4):
                nc.tensor.matmul(
                    aff[:, 128 * j:128 * (j + 1)],
                    lhsT=bc4[:, 32 * j:32 * (j + 1)],
                    rhs=m4[:, :],
                    start=False, stop=True,
                )
            out_sb = sb.tile([B, D], f32, name="out_sb")
            nc.vector.tensor_add(out=out_sb[:, :], in0=aff[:, :], in1=z_sb[:, :])
            nc.scalar.dma_start(out=out_ext[:, :], in_=out_sb[:, :])

    # patch the real remote-sem wait values in (see comment at emission)
    for ins, semname, val in patch_waits:
        si = ins.sync_info
        found = False
        for w in si.on_wait:
            if w.ant_name == semname:
                w.wait_value = val
                found = True
        assert found, (semname, si)
        ins.sync_info = si

    nc.finalize()
    return nc


_NC_CACHE = {}


def _get_nc():
    if "nc" not in _NC_CACHE:
        _NC_CACHE["nc"] = _build_nc()
    return _NC_CACHE["nc"]


def _in_maps(x, z):
    zt = np.ascontiguousarray(z.T)
    return [
        {"x": np.ascontiguousarray(x[c * BC:(c + 1) * BC]), "z": z, "zt": zt}
        for c in range(NCORES)
    ]


def kernel(x: np.ndarray, z: np.ndarray) -> np.ndarray:
    from concourse.bass_utils import run_bass_kernel_spmd

    x = np.ascontiguousarray(np.asarray(x, dtype=np.float32))
    z = np.ascontiguousarray(np.asarray(z, dtype=np.float32))
    nc = _get_nc()
    res = run_bass_kernel_spmd(nc, _in_maps(x, z), core_ids=list(range(NCORES)))
    return np.asarray(res.results[0]["out"], dtype=np.float32)


# revision 28
# speedup vs baseline: 1.1709x; 1.1160x over previous
"""Trainium2 Bass kernel for nn_BiasVectorsBlock (MVN sampling block).

Computes, for x [32, 2048, 512] and z [32, 512]:
    mean = mean(x, axis=(0,1))
    cov  = mean_b( xc_b^T xc_b / (T-1) ),  xc_b = x_b - mean_t(x_b)
    L    = cholesky(cov);  out = mean + z @ L^T

Strategy (8 NeuronCores, data-parallel over B):
  - core c DMA-loads its 4 batches once (f32), casts to bf16 on
    DVE/ScalarE.  TensorE accumulates upper-triangle Gram strips in
    PSUM across all 64 chunks, plus tiny per-strip indicator matmuls
    (rhs = one-hot batch columns) that give the per-batch column sums
    s_b on PE for free — no DVE fold chains.
  - tail: transpose s blocks via eye-matmuls, apply the -s_b s_b^T/T
    corrections, subtract SHIFT*I, pack upper strips + transposed mean
    into a [128, 1288] bf16 payload.
  - AllReduce = hand-rolled XOR recursive doubling over remote_dma
    SBUF->SBUF broadcasts (relative dests, so the SPMD program is
    uniform): 3 stages (partner = me ^ 2^k), each stage sends the
    current partial (sliced across dest slots so all DMA engines run),
    waits for the partner's payload, adds it on DVE.  Replaces the
    ncfw collective (24us + 11.5us trigger delay) with ~10us.
  - every core then runs the sqrt-free Cholesky fixed-point rounds
    (Y <- Phi_u(E - Y^T Y), 2 rounds) and the affine out = z + z@Y +
    mean; core 0's output is the result.
"""

import os
import sys

for _p in ("/opt/trn_rl_repo",):
    if _p not in sys.path and os.path.isdir(_p):
        sys.path.insert(0, _p)

import numpy as np

B, T, D = 32, 2048, 512
NCORES = 8
BC = B // NCORES          # batches per core
CH = T // 128             # 128-row chunks per batch
DENOM = (T - 1) * B       # cov denominator
SHIFT = DENOM / NCORES    # identity shift per core, so payload is zero-mean
W = [512, 384, 256, 128]  # upper-strip widths (strip i: rows 128i.., cols 128i..512)
OFFS = [0, 512, 896, 1152]  # packed strip offsets
PAY = 1288                # payload cols: 1280 strips + 4 mean(T) + 4 pad
NSL = PAY // 4            # 322 cols (644 B) per slice


def _build_nc():
    import concourse.bacc as bacc
    import concourse.mybir as mybir
    import ml_dtypes
    from bass_rust import InstructionNameOrderedSet
    from concourse import library_config
    from concourse.tile import TileContext

    f32 = mybir.dt.float32
    bf16 = mybir.dt.bfloat16
    mult = mybir.AluOpType.mult

    nc = bacc.Bacc(None, num_devices=NCORES)

    x_in = nc.declare_dram_parameter("x", [BC, T, D], f32, isOutput=False)
    z_in = nc.declare_dram_parameter("z", [B, D], f32, isOutput=False)
    zt_in = nc.declare_dram_parameter("zt", [D, B], f32, isOutput=False)
    out_ext = nc.declare_dram_parameter("out", [B, D], f32, isOutput=True)

    # ---- constants (embedded in the NEFF) ----
    # -Phi mask: local cols 0:128 = diagonal block (strict-upper -> -1,
    # diag -> -0.5, lower -> 0); cols 128:512 -> -1.
    m = np.zeros((128, 512), np.float32)
    m[:, 128:] = -1.0
    r, c = np.indices((128, 128))
    m[:, :128] = np.where(c > r, -1.0, np.where(c == r, -0.5, 0.0)).astype(np.float32)
    maskneg_d = nc.inline_tensor(m, name="maskneg")
    maskpd_d = nc.inline_tensor(-m * (2.0 ** -16), name="maskpd")

    eye = np.eye(128, dtype=np.float32)
    eyeb_d = nc.inline_tensor((-eye * 2.0 ** -16).astype(ml_dtypes.bfloat16), name="eyeb")
    eye128b_d = nc.inline_tensor(eye.astype(ml_dtypes.bfloat16), name="eye128b")
    negshifti_d = nc.inline_tensor((-SHIFT) * eye, name="negshifti")
    # indicator columns: col 4b+j = 1 iff j == b (slice [:, 4b:4b+4] per batch)
    ind = np.zeros((128, 4 * BC), np.float32)
    for b in range(BC):
        ind[:, 4 * b + b] = 1.0
    ind16_d = nc.inline_tensor(ind.astype(ml_dtypes.bfloat16), name="ind16")
    ones4x1_d = nc.inline_tensor(np.ones((BC, 1), ml_dtypes.bfloat16), name="ones4x1")
    # mean-broadcast selectors: bc4[k, 32j+b] = 2^-16 iff k == j
    bc = np.zeros((BC, 4 * B), np.float32)
    for j in range(4):
        bc[j, 32 * j:32 * (j + 1)] = 2.0 ** -16
    bc4_d = nc.inline_tensor(bc.astype(ml_dtypes.bfloat16), name="bc4")

    with TileContext(nc) as tc, \
            tc.tile_pool(name="sb", bufs=1) as sb, \
            tc.tile_pool(name="dr", space="DRAM", bufs=1) as dr:

        # payload staging: acc0 is the packed local partial; acc3 receives
        # the allreduced result for phase B.
        accs = [sb.tile([128, PAY], bf16, name=f"acc{k}") for k in (0, 3)]
        accs = {0: accs[0], 3: accs[1]}

        # ---- phase A: Gram strips + per-batch column sums ----
        with tc.tile_pool(name="psA", space="PSUM", bufs=1) as ps:
            g = [ps.tile([128, W[i]], f32, tag=f"g{i}", bufs=1, name=f"g{i}")
                 for i in range(4)]
            sblk16 = ps.tile([128, 4 * BC], f32, tag="sb16", bufs=1, name="sblk16")
            sblk = [sblk16[:, 4 * i:4 * (i + 1)] for i in range(4)]
            for b in range(BC):
                xf = sb.tile([128, CH * D], f32, tag="xf", bufs=4, name=f"xf{b}")
                xb = sb.tile([128, CH * D], bf16, tag="xb", bufs=2, name=f"xb{b}")
                xf3 = xf.rearrange("p (c d) -> p c d", d=D)
                xs3 = x_in[b].rearrange("(c p) d -> p c d", p=128)
                if b == 0:
                    c0 = 0
                    for span in (2, 2, 4, 4, 4):
                        nc.sync.dma_start(out=xf3[:, c0:c0 + span, :],
                                          in_=xs3[:, c0:c0 + span, :])
                        c0 += span
                else:
                    half = CH // 2
                    nc.sync.dma_start(out=xf3[:, :half, :], in_=xs3[:, :half, :])
                    nc.sync.dma_start(out=xf3[:, half:, :], in_=xs3[:, half:, :])
                if b == 0:
                    # only ind16 (needed by the first chunk matmuls) and
                    # z/zt load now; the big masks wait until the x DMA
                    # stream is wound down (b == 2) so they don't steal
                    # HBM bandwidth from the critical path.
                    ind16 = sb.tile_from(ind16_d[:, :], name="ind16_sb", forced_dma_engine=mybir.EngineType.Activation)
                    z_sb = sb.tile([B, D], f32, name="z_sb")
                    nc.scalar.dma_start(out=z_sb[:, :], in_=z_in[:, :])
                    zts = []
                    for k in range(4):
                        zt_k = sb.tile([128, B], f32, name=f"zt{k}_sb")
                        nc.scalar.dma_start(out=zt_k[:, :],
                                            in_=zt_in[k * 128:(k + 1) * 128, :])
                        ztb_k = sb.tile([128, B], bf16, name=f"ztb{k}_sb")
                        nc.vector.tensor_copy(out=ztb_k[:, :], in_=zt_k[:, :])
                        zts.append(ztb_k)
                    nc.vector.memset(accs[0][:, 1284:PAY], 0.0)
                if b == 2:
                    maskneg = sb.tile_from(maskneg_d[:, :], name="maskneg_sb", forced_dma_engine=mybir.EngineType.Activation)
                    maskpd = sb.tile_from(maskpd_d[:, :], name="maskpd_sb", forced_dma_engine=mybir.EngineType.Activation)
                    eyeb = sb.tile_from(eyeb_d[:, :], name="eyeb_sb", forced_dma_engine=mybir.EngineType.Activation)
                    eye128b = sb.tile_from(eye128b_d[:, :], name="eye128b_sb", forced_dma_engine=mybir.EngineType.Activation)
                    negshifti = sb.tile_from(negshifti_d[:, :], name="negshifti_sb", forced_dma_engine=mybir.EngineType.Activation)
                    ones4x1 = sb.tile_from(ones4x1_d[:, :], name="ones4x1_sb", forced_dma_engine=mybir.EngineType.Activation)
                    bc4 = sb.tile_from(bc4_d[:, :], name="bc4_sb", forced_dma_engine=mybir.EngineType.Activation)
                # casts alternate DVE/ScalarE
                if b == 0:
                    c0 = 0
                    for pi, span in enumerate((2, 2, 4, 4, 4)):
                        so = xb[:, c0 * D:(c0 + span) * D]
                        si = xf[:, c0 * D:(c0 + span) * D]
                        if pi % 2 == 0:
                            nc.vector.tensor_copy(out=so, in_=si)
                        else:
                            nc.scalar.copy(out=so, in_=si)
                        c0 += span
                else:
                    q = CH * D // 4
                    for qi in range(4):
                        so = xb[:, qi * q:(qi + 1) * q]
                        si = xf[:, qi * q:(qi + 1) * q]
                        if qi % 2 == 0:
                            nc.vector.tensor_copy(out=so, in_=si)
                        else:
                            nc.scalar.copy(out=so, in_=si)
                first = b == 0
                last = b == BC - 1
                for cch in range(CH):
                    xc = xb[:, cch * D:(cch + 1) * D]
                    st = first and cch == 0
                    sp = last and cch == CH - 1
                    for i in range(4):
                        nc.tensor.matmul(
                            g[i][:, :],
                            lhsT=xc[:, i * 128:(i + 1) * 128],
                            rhs=xc[:, 128 * i:],
                            start=st, stop=False,
                        )
                        nc.tensor.matmul(
                            sblk[i],
                            lhsT=xc[:, i * 128:(i + 1) * 128],
                            rhs=ind16[:, 4 * b:4 * (b + 1)],
                            start=st, stop=sp,
                        )

            # ---- tail: s extraction, corrections, mean, pack ----
            sblk_sb = []
            for i in range(4):
                t = sb.tile([128, BC], bf16, name=f"sblk_sb{i}")
                if i % 2 == 0:
                    nc.vector.tensor_copy(out=t[:, :], in_=sblk[i])
                else:
                    nc.scalar.copy(out=t[:, :], in_=sblk[i])
                sblk_sb.append(t)
            srowT = ps.tile([BC, D], f32, tag="srowT", bufs=1, name="srowT")
            for i in range(4):
                nc.tensor.matmul(
                    srowT[:, 128 * i:128 * (i + 1)],
                    lhsT=sblk_sb[i][:, :], rhs=eye128b[:, :],
                    start=True, stop=True,
                )
            s_bf = sb.tile([BC, D], bf16, name="s_bf")
            nc.vector.tensor_copy(out=s_bf[:, :], in_=srowT[:, :])
            sneg = sb.tile([BC, D], bf16, name="sneg")
            nc.vector.tensor_scalar_mul(sneg[:, :], srowT[:, :], -1.0 / T)
            for i in range(4):
                nc.tensor.matmul(
                    g[i][:, :],
                    lhsT=sneg[:, i * 128:(i + 1) * 128],
                    rhs=s_bf[:, 128 * i:],
                    start=False, stop=True,
                )
            # transposed mean columns: mc[:, j] = sum_b s_b[128j:128(j+1)]
            mc = ps.tile([128, 4], f32, tag="mc", bufs=1, name="mc")
            for j in range(4):
                nc.tensor.matmul(
                    mc[:, j:j + 1],
                    lhsT=s_bf[:, 128 * j:128 * (j + 1)], rhs=ones4x1[:, :],
                    start=True, stop=True,
                )
            # pack into acc0
            pack_ops = []
            for i in range(4):
                pack_ops.append(nc.vector.tensor_add(
                    out=accs[0][:, OFFS[i]:OFFS[i] + 128],
                    in0=g[i][:, 0:128],
                    in1=negshifti[:, :],
                ))
                if W[i] > 128:
                    pack_ops.append(nc.scalar.copy(
                        out=accs[0][:, OFFS[i] + 128:OFFS[i] + W[i]],
                        in_=g[i][:, 128:W[i]],
                    ))
            pack_ops.append(nc.vector.tensor_copy(out=accs[0][:, 1280:1284],
                                                  in_=mc[:, :]))


        # ---- AllReduce: XOR recursive doubling over remote_dma ----
        # Trigger k waits on psem k (incremented by an explicit sem_inc
        # ordered after the producers of acc_k).  The rsem waits sit
        # directly on the adds and are emitted with value 0 so Tile's
        # single-core scheduling sim (which cannot see the partner's
        # remote sem updates) does not flag a deadlock; the real wait
        # values are patched in after scheduling, before finalize.
        # ---- AllReduce via ncfw (RDH): SBUF -> DRAM -> CC -> DRAM -> SBUF
        ar_in = dr.tile([128, PAY], bf16, name="ar_in")
        ar_out = dr.tile([128, PAY], bf16, addr_space="Shared", name="ar_out")
        nc.scalar.dma_start(out=ar_in[:, :], in_=accs[0][:, :])
        nc.gpsimd.collective_compute(
            "AllReduce",
            mybir.AluOpType.add,
            replica_groups=[list(range(NCORES))],
            ins=[ar_in[:, :].opt()],
            outs=[ar_out[:, :].opt()],
        )
        nc.scalar.dma_start(out=accs[3][:, :], in_=ar_out[:, :])
        patch_waits = []

        # ---- phase B: Cholesky fixed-point iteration + affine ----
        acc3 = accs[3]
        with tc.tile_pool(name="psB", space="PSUM", bufs=1) as ps:
            ebn_raw = [acc3[:, OFFS[i]:OFFS[i] + W[i]] for i in range(4)]
            # round 0: Y = Phi(E) = raw * (mask/DENOM)
            Y = []
            for i in range(4):
                y0 = sb.tile([128, W[i]], bf16, tag="y", bufs=8, name=f"y0_{i}")
                nc.vector.tensor_tensor(out=y0[:, :], in0=ebn_raw[i],
                                        in1=maskpd[:, :W[i]], op=mult)
                Y.append(y0)
            # round 1: Y <- Phi(E - Y^T Y)
            newY = []
            for i in range(4):
                p = ps.tile([128, W[i]], f32, tag="it", bufs=4, name=f"it1_{i}")
                first = True
                for k in range(i + 1):
                    lo = 128 * (i - k)
                    nc.tensor.matmul(
                        p[:, :],
                        lhsT=Y[k][:, lo:lo + 128],
                        rhs=Y[k][:, lo:],
                        start=first, stop=False,
                    )
                    first = False
                nc.tensor.matmul(p[:, :], lhsT=eyeb[:, :], rhs=ebn_raw[i],
                                 start=first, stop=True)
                ny = sb.tile([128, W[i]], bf16, tag="y", bufs=8, name=f"y1_{i}")
                nc.vector.tensor_tensor(out=ny[:, :], in0=p[:, :],
                                        in1=maskneg[:, :W[i]], op=mult)
                newY.append(ny)
            Y = newY

            # affine: out = z + z @ Y + mean
            aff = ps.tile([B, D], f32, tag="aff", bufs=1, name="aff")
            for k in range(4):
                nc.tensor.matmul(
                    aff[:, 128 * k:],
                    lhsT=zts[k][:, :],
                    rhs=Y[k][:, :],
                    start=(k == 0), stop=False,
                )
            # mean: transpose mc columns back to a [4, 128] row block, then
            # broadcast to all 32 output rows (selector consts carry 1/(B*T)).
            mrowT = ps.tile([BC, 128], f32, tag="mrowT", bufs=1, name="mrowT")
            nc.tensor.matmul(mrowT[:, :], lhsT=acc3[:, 1280:1284],
                             rhs=eye128b[:, :], start=True, stop=True)
            m4 = sb.tile([BC, 128], bf16, name="m4")
            nc.vector.tensor_copy(out=m4[:, :], in_=mrowT[:, :])
            for j in range(4):
                nc.tensor.matmul(
                    aff[:, 128 * j:128 * (j + 1)],
                    lhsT=bc4[:, 32 * j:32 * (j + 1)],
                    rhs=m4[:, :],
                    start=False, stop=True,
                )
            out_sb = sb.tile([B, D], f32, name="out_sb")
            nc.vector.tensor_add(out=out_sb[:, :], in0=aff[:, :], in1=z_sb[:, :])
            nc.scalar.dma_start(out=out_ext[:, :], in_=out_sb[:, :])

    # patch the real remote-sem wait values in (see comment at emission)
    for ins, semname, val in patch_waits:
        si = ins.sync_info
        found = False
        for w in si.on_wait:
            if w.ant_name == semname:
                w.wait_value = val
                found = True
        assert found, (semname, si)
        ins.sync_info = si

    nc.finalize()
    return nc


_NC_CACHE = {}


def _get_nc():
    if "nc" not in _NC_CACHE:
        _NC_CACHE["nc"] = _build_nc()
    return _NC_CACHE["nc"]


def _in_maps(x, z):
    zt = np.ascontiguousarray(z.T)
    return [
        {"x": np.ascontiguousarray(x[c * BC:(c + 1) * BC]), "z": z, "zt": zt}
        for c in range(NCORES)
    ]


def kernel(x: np.ndarray, z: np.ndarray) -> np.ndarray:
    from concourse.bass_utils import run_bass_kernel_spmd

    x = np.ascontiguousarray(np.asarray(x, dtype=np.float32))
    z = np.ascontiguousarray(np.asarray(z, dtype=np.float32))
    nc = _get_nc()
    res = run_bass_kernel_spmd(nc, _in_maps(x, z), core_ids=list(range(NCORES)))
    return np.asarray(res.results[0]["out"], dtype=np.float32)
